# revision 1
# baseline (speedup 1.0000x reference)
"""Blockwise (compressed-KV) attention on 8 Trainium2 NeuronCores.

Problem: q,k,v [B=4,H=16,T=4096,D=128] fp32, BS=32.
  k_cmp/v_cmp = blockwise mean-pool of k/v along T -> [B,H,C=128,D]
  score = softmax(q @ k_cmp^T / sqrt(D))   [B,H,T,C]
  out   = score @ v_cmp                    [B,H,T,D]
Returns (out, score), matching the reference.

Sharding: the 64 (b,h) pairs are split 8-per-core (pure data parallel, no
communication).  Each core runs an identical Bass/Tile program over its
8 heads.

Per-head dataflow on one core (all fp32):
  compression: 32 chunk matmuls per tensor with the k/v chunk [t,d] as the
    stationary operand and a [128,4] pooling matrix (1/32 blocks) as the
    moving operand -> k_cmp^T and v_cmp^T [d,c] accumulate in one PSUM bank
    (disjoint 4-column slices, one single-matmul group each).
  main loop over 8 supertiles of 512 q rows:
    PE-transpose 4x [128,128] q tiles into one PSUM bank, evacuate once,
    QK^T as 4 matmuls (q^T tile stationary, k_cmp^T moving),
    exp via ScalarE (scale=1/sqrt(D) folded in) PSUM->SBUF,
    row sums via one segmented VectorE reduce, reciprocal,
    normalize on GpSimd (per-partition scalar), store score,
    PE-transpose the normalized score tiles, evacuate, PV as 4 matmuls
    (score^T tile stationary, v_cmp moving), evacuate via ScalarE, store out.
"""
import math

import numpy as np

import concourse.bass as bass
import concourse.tile as tile
from concourse import mybir
from concourse.bass_utils import run_bass_kernel_spmd
from concourse.vector_clock import ScopedClock

B, H, T, D = 4, 16, 4096, 128
BS_EXPECTED = 32
C = T // BS_EXPECTED  # 128 compressed slots
N_CORES = 8
HEADS_PER_CORE = B * H // N_CORES  # 8
N_SUPER = T // 512  # 8 supertiles of 512 rows per head
F32 = mybir.dt.float32

# ---------------------------------------------------------------------------
# walrus in this toolchain rejects instructions carrying more than one sync
# wait.  Tile's scheduler freely emits several waits per instruction, and the
# kernel-tail drain accumulates one wait per outstanding semaphore.  Hoist all
# but one wait of every instruction onto dedicated same-engine NOPs placed
# immediately before it (same-engine program order keeps the semantics).
_MAX_WAITS = 1
_split_counter = [0]


def _split_multi_waits(ordered):
    for insts in ordered.values():
        expanded = []
        for inst in insts:
            si = inst.sync_info
            if si is not None and len(si.on_wait) > _MAX_WAITS:
                waits = list(si.on_wait)
                head, keep = waits[:-_MAX_WAITS], waits[-_MAX_WAITS:]
                for w in head:
                    _split_counter[0] += 1
                    expanded.append(mybir.InstNoOp(
                        name=f"waitsplit_{_split_counter[0]}",
                        ins=[], outs=[],
                        engine=inst.engine,
                        sync_info=mybir.SyncInfo(on_wait=[w], on_update=[]),
                        bass_nofuse=True,
                    ))
                inst.sync_info = mybir.SyncInfo(
                    on_wait=keep, on_update=list(si.on_update)
                )
            expanded.append(inst)
        insts[:] = expanded


_orig_lower_ordered = tile.TileContext._lower_ordered_insts


def _lower_ordered_split(self, ordered):
    _split_multi_waits(ordered)
    return _orig_lower_ordered(self, ordered)


tile.TileContext._lower_ordered_insts = _lower_ordered_split


def _drain_and_barrier_split(self, tick_clock, wait_clock):
    nc = self.nc
    drain_inst = nc.sync.drain()
    wait_clock.add_sem_waits(
        drain_inst.ins, ScopedClock({None: tick_clock.global_clock})
    )
    si = drain_inst.ins.sync_info
    waits = list(si.on_wait) if si is not None else []
    if len(waits) > _MAX_WAITS:
        drain_inst.ins.sync_info = mybir.SyncInfo(
            on_wait=waits[:_MAX_WAITS], on_update=list(si.on_update)
        )
        for i in range(_MAX_WAITS, len(waits), _MAX_WAITS):
            extra = nc.sync.drain()
            extra.ins.sync_info = mybir.SyncInfo(
                on_wait=waits[i : i + _MAX_WAITS], on_update=[]
            )
    nc.all_engine_barrier()
    assert self.sems is not None
    popped = nc._tile_sem_poison_stack.pop()
    assert popped is self._sem_poison
    nc.clear_and_free_semaphores(list(self.sems.allocated().values()))
    nc.all_engine_barrier()


tile.TileContext._drain_and_barrier = _drain_and_barrier_split
# ---------------------------------------------------------------------------


def build_program(reps: int = 1, norm_engine: str = "dve",
                  q_f32_transpose: bool = True, pe_sums: bool = False,
                  big_bufs: bool = True, dma_only: bool = False) -> bass.Bass:
    """Build the per-core Bass program.  `reps` repeats the whole computation
    (identical work, same outputs) for slope-based wall-clock timing.

    v2: 1 MiB DMAs (loads on the SP HWDGE ring, stores on the ACT ring),
    bf16 transposes + QK/PV matmuls (compression and softmax stay fp32)."""
    BF16 = mybir.dt.bfloat16
    nc = bass.Bass("TRN2", target_bir_lowering=False, debug=False,
                   num_devices=N_CORES)

    q_d = nc.dram_tensor("q", [HEADS_PER_CORE, T, D], F32, kind="ExternalInput").ap()
    k_d = nc.dram_tensor("k", [HEADS_PER_CORE, T, D], F32, kind="ExternalInput").ap()
    v_d = nc.dram_tensor("v", [HEADS_PER_CORE, T, D], F32, kind="ExternalInput").ap()
    ident_d = nc.dram_tensor("ident", [128, 128], F32, kind="ExternalInput").ap()
    pmat_d = nc.dram_tensor("pmat", [128, 4], F32, kind="ExternalInput").ap()
    out_d = nc.dram_tensor("out", [HEADS_PER_CORE, T, D], F32,
                           kind="ExternalOutput").ap()
    score_d = nc.dram_tensor("score", [HEADS_PER_CORE, T, C], F32,
                             kind="ExternalOutput").ap()

    inv_sqrt_d = 1.0 / math.sqrt(D)

    with tile.TileContext(nc) as tc:
        with (
            tc.tile_pool(name="singles", bufs=1) as singles,
            tc.tile_pool(name="kv", bufs=4 if big_bufs else 3) as kv_pool,
            tc.tile_pool(name="heads", bufs=2) as heads,
            tc.tile_pool(name="qsb", bufs=4 if big_bufs else 3) as qsb_pool,
            tc.tile_pool(name="sb", bufs=6 if big_bufs else 4) as sb_pool,
            tc.tile_pool(name="stream", bufs=4 if big_bufs else 3) as stream_pool,
            tc.tile_pool(name="small", bufs=4) as small_pool,
            tc.tile_pool(name="psA", bufs=2, space="PSUM") as psA,
            tc.tile_pool(name="psS", bufs=2, space="PSUM") as psS,
            tc.tile_pool(name="psST", bufs=2, space="PSUM") as psST,
            tc.tile_pool(name="psO", bufs=2, space="PSUM") as psO,
        ):
            ident = singles.tile([128, 128], F32)
            nc.sync.dma_start(out=ident, in_=ident_d)
            ident_bf = singles.tile([128, 128], BF16)
            nc.vector.tensor_copy(ident_bf, ident)
            pmat = singles.tile([128, 4], F32)
            nc.sync.dma_start(out=pmat, in_=pmat_d)
            ones_bf = singles.tile([128, 1], BF16)
            nc.vector.memset(ones_bf, 1.0)

            for _rep in range(reps):
                for h in range(HEADS_PER_CORE):
                    # ---- compression (fp32): k_cmp^T | v_cmp^T in one bank
                    kcvc = psST.tile([128, 512], F32, tag="st")
                    for half in range(2):  # 1 MiB k/v loads
                        rows = slice(half * 2048, (half + 1) * 2048)
                        k_sb = kv_pool.tile([128, 16, D], F32, tag="kv")
                        nc.sync.dma_start(
                            out=k_sb,
                            in_=k_d[h, rows, :].rearrange("(j p) d -> p j d", p=128),
                        )
                        v_sb = kv_pool.tile([128, 16, D], F32, tag="kv")
                        nc.sync.dma_start(
                            out=v_sb,
                            in_=v_d[h, rows, :].rearrange("(j p) d -> p j d", p=128),
                        )
                        for j in range(16 if not dma_only else 0):
                            cc = 16 * half + j  # chunk index 0..31
                            nc.tensor.matmul(
                                kcvc[:, 4 * cc : 4 * cc + 4],
                                lhsT=k_sb[:, j, :], rhs=pmat,
                                start=True, stop=True,
                            )
                            nc.tensor.matmul(
                                kcvc[:, 256 + 4 * cc : 256 + 4 * cc + 4],
                                lhsT=v_sb[:, j, :], rhs=pmat,
                                start=True, stop=True,
                            )
                    if dma_only:
                        for gg in range(N_SUPER // 2):
                            prow = slice(gg * 1024, (gg + 1) * 1024)
                            q_sb = qsb_pool.tile([128, 8, D], F32, tag="q")
                            nc.sync.dma_start(
                                out=q_sb,
                                in_=q_d[h, prow, :].rearrange(
                                    "(j p) d -> p j d", p=128),
                            )
                            score_pair = stream_pool.tile(
                                [128, 8, C], F32, tag="score")
                            nc.vector.memset(score_pair[:, 0:1, 0:1], 0.5)
                            out_pair = stream_pool.tile(
                                [128, 8, D], F32, tag="out")
                            nc.vector.memset(out_pair[:, 0:1, 0:1], 0.25)
                            nc.scalar.dma_start(
                                out=score_d[h, prow, :].rearrange(
                                    "(j p) c -> p j c", p=128),
                                in_=score_pair,
                            )
                            nc.scalar.dma_start(
                                out=out_d[h, prow, :].rearrange(
                                    "(j p) d -> p j d", p=128),
                                in_=out_pair,
                            )
                        continue
                    k_cmpT = heads.tile([128, C], BF16, tag="kc")  # [d, c] bf16
                    nc.scalar.copy(k_cmpT, kcvc[:, 0:128])
                    v_cmpT = heads.tile([128, C], F32, tag="vt")  # [d, c] f32
                    nc.scalar.copy(v_cmpT, kcvc[:, 256:384])
                    vps = psO.tile([128, 512], F32, tag="o")
                    nc.tensor.transpose(vps[:, 0:128], v_cmpT, ident)
                    v_cmp = heads.tile([128, D], BF16, tag="vc")  # [c, d] bf16
                    nc.scalar.copy(v_cmp, vps[:, 0:128])

                    # ---- main loop: 4 pairs of supertiles (1 MiB q/score/out)
                    for gg in range(N_SUPER // 2):
                        prow = slice(gg * 1024, (gg + 1) * 1024)
                        q_sb = qsb_pool.tile([128, 8, D], F32, tag="q")
                        nc.sync.dma_start(
                            out=q_sb,
                            in_=q_d[h, prow, :].rearrange("(j p) d -> p j d", p=128),
                        )
                        score_pair = stream_pool.tile([128, 8, C], F32, tag="score")
                        out_pair = stream_pool.tile([128, 8, D], F32, tag="out")

                        for sub in range(2):
                            cols = slice(sub * 512, (sub + 1) * 512)
                            if q_f32_transpose:
                                qT_ps = psA.tile([128, 512], F32, tag="a")
                                for j in range(4):
                                    nc.tensor.transpose(
                                        qT_ps[:, 128 * j : 128 * (j + 1)],
                                        q_sb[:, 4 * sub + j, :], ident,
                                    )
                            else:
                                q_bf = sb_pool.tile([128, 512], BF16, tag="qbf")
                                nc.vector.tensor_copy(
                                    q_bf,
                                    q_sb.rearrange("p j d -> p (j d)")[:, cols],
                                )
                                qT_ps = psA.tile([128, 512], BF16, tag="a")
                                for j in range(4):
                                    nc.tensor.transpose(
                                        qT_ps[:, 128 * j : 128 * (j + 1)],
                                        q_bf[:, 128 * j : 128 * (j + 1)],
                                        ident_bf,
                                    )
                            qT = sb_pool.tile([128, 512], BF16, tag="qT")
                            nc.vector.tensor_copy(qT, qT_ps)

                            s_ps = psS.tile([128, 512], F32, tag="s")
                            for j in range(4):
                                nc.tensor.matmul(
                                    s_ps[:, 128 * j : 128 * (j + 1)],
                                    lhsT=qT[:, 128 * j : 128 * (j + 1)],
                                    rhs=k_cmpT,
                                    start=True, stop=True,
                                )
                            # S^T [c, t] in one matmul: k_cmp^T stationary,
                            # q^T streaming N=512
                            stp_ps = psST.tile([128, 512], F32, tag="st")
                            nc.tensor.matmul(
                                stp_ps, lhsT=k_cmpT, rhs=qT,
                                start=True, stop=True,
                            )
                            expt = sb_pool.tile([128, 512], F32, tag="exp")
                            nc.scalar.activation(
                                expt, s_ps, mybir.ActivationFunctionType.Exp,
                                scale=inv_sqrt_d,
                            )
                            # exp(S^T) straight to bf16 SBUF = PV weights
                            expT_bf = sb_pool.tile([128, 512], BF16, tag="st")
                            nc.scalar.activation(
                                expT_bf, stp_ps, mybir.ActivationFunctionType.Exp,
                                scale=inv_sqrt_d,
                            )
                            recip = small_pool.tile([128, 4], F32, tag="recip")
                            if pe_sums:
                                sums_ps = psST.tile([128, 4], F32, tag="st")
                                for j in range(4):
                                    nc.tensor.matmul(
                                        sums_ps[:, j : j + 1],
                                        lhsT=expT_bf[:, 128 * j : 128 * (j + 1)],
                                        rhs=ones_bf,
                                        start=True, stop=True,
                                    )
                                nc.vector.reciprocal(recip, sums_ps)
                            else:
                                sums = small_pool.tile([128, 4], F32, tag="sums")
                                nc.vector.reduce_sum(
                                    sums,
                                    expt.rearrange("p (j c) -> p j c", j=4),
                                    axis=mybir.AxisListType.X,
                                )
                                nc.vector.reciprocal(recip, sums)

                            score_half = score_pair.rearrange(
                                "p j c -> p (j c)")[:, cols]
                            norm_eng = (nc.gpsimd if norm_engine == "pool"
                                        else nc.vector)
                            for j in range(4):
                                norm_eng.tensor_scalar_mul(
                                    score_half[:, 128 * j : 128 * (j + 1)],
                                    expt[:, 128 * j : 128 * (j + 1)],
                                    recip[:, j : j + 1],
                                )

                            # PV on unnormalized exp^T; fold 1/rowsum into the
                            # PSUM evacuation (split ACT / DVE)
                            o_ps = psO.tile([128, 512], F32, tag="o")
                            for j in range(4):
                                nc.tensor.matmul(
                                    o_ps[:, 128 * j : 128 * (j + 1)],
                                    lhsT=expT_bf[:, 128 * j : 128 * (j + 1)],
                                    rhs=v_cmp,
                                    start=True, stop=True,
                                )
                            out_half = out_pair.rearrange(
                                "p j d -> p (j d)")[:, cols]
                            for j in range(4):
                                if j < 2:
                                    nc.scalar.activation(
                                        out_half[:, 128 * j : 128 * (j + 1)],
                                        o_ps[:, 128 * j : 128 * (j + 1)],
                                        mybir.ActivationFunctionType.Copy,
                                        scale=recip[:, j : j + 1],
                                    )
                                else:
                                    nc.vector.tensor_scalar_mul(
                                        out_half[:, 128 * j : 128 * (j + 1)],
                                        o_ps[:, 128 * j : 128 * (j + 1)],
                                        recip[:, j : j + 1],
                                    )

                        # 1 MiB stores on the ACT HWDGE ring
                        nc.scalar.dma_start(
                            out=score_d[h, prow, :].rearrange(
                                "(j p) c -> p j c", p=128),
                            in_=score_pair,
                        )
                        nc.scalar.dma_start(
                            out=out_d[h, prow, :].rearrange(
                                "(j p) d -> p j d", p=128),
                            in_=out_pair,
                        )
    return nc


def _make_const_inputs():
    ident = np.eye(128, dtype=np.float32)
    pmat = np.zeros((128, 4), dtype=np.float32)
    for t in range(128):
        pmat[t, t // 32] = 1.0 / 32.0
    return ident, pmat


_PROGRAM_CACHE: dict[int, bass.Bass] = {}


def kernel(q: np.ndarray, k: np.ndarray, v: np.ndarray, BS) -> tuple:
    assert int(BS) == BS_EXPECTED, f"kernel hardcodes BS=32, got {BS}"
    q = np.ascontiguousarray(np.asarray(q, dtype=np.float32)).reshape(B * H, T, D)
    k = np.ascontiguousarray(np.asarray(k, dtype=np.float32)).reshape(B * H, T, D)
    v = np.ascontiguousarray(np.asarray(v, dtype=np.float32)).reshape(B * H, T, D)

    if 1 not in _PROGRAM_CACHE:
        _PROGRAM_CACHE[1] = build_program(reps=1)
    nc = _PROGRAM_CACHE[1]

    ident, pmat = _make_const_inputs()
    in_maps = []
    for i in range(N_CORES):
        sl = slice(i * HEADS_PER_CORE, (i + 1) * HEADS_PER_CORE)
        in_maps.append({
            "q": q[sl], "k": k[sl], "v": v[sl],
            "ident": ident, "pmat": pmat,
        })

    res = run_bass_kernel_spmd(nc, in_maps, core_ids=list(range(N_CORES)))

    out = np.empty((B * H, T, D), dtype=np.float32)
    score = np.empty((B * H, T, C), dtype=np.float32)
    for i in range(N_CORES):
        sl = slice(i * HEADS_PER_CORE, (i + 1) * HEADS_PER_CORE)
        out[sl] = res.results[i]["out"]
        score[sl] = res.results[i]["score"]
    return out.reshape(B, H, T, D), score.reshape(B, H, T, C)



# revision 2
# speedup vs baseline: 4.1217x; 4.1217x over previous
"""Blockwise (compressed-KV) attention on 8 Trainium2 NeuronCores.

Per-core dataflow (8 heads/core, all HBM I/O bf16):
  q:   DMA-transpose load (xbar) -> qT [d, t] in SBUF, true t order.
  k,v: contiguous loads [p=t//32, a=t%32, d] (8 KiB runs); compression via
       identity-stationary accumulating matmuls: cmp[c,d] = sum_a x[:,a,:].
       v_cmp evacuated with 1/32 scale (bf16); k side transposed once to
       k_cmpT [d,c] (1/32 folded into the exp scale).
  per 512-row sub:
       S[t,c]: 4 matmuls (qT tile stationary, k_cmpT moving)
       exp on ACT -> score slots (bf16, unnormalized)
       row sums: DVE segmented reduce; reciprocal on DVE
       normalize score in place (POOL or DVE)
       score^T: 4 PE transposes of the normalized slots -> PSUM (bf16)
       evac -> w [c,t] bf16;  PV: ONE matmul out^T[d,t] += v_cmp^T w
       (v_cmp stationary per head, w moving N=512); evac out^T -> bf16
  stores (SWDGE/gpsimd ring): score as [p, j, C] blocks, out as out^T
       [D, T]; the host undoes both layouts (pure reshape/transpose).
"""
import math

import numpy as np

import concourse.bass as bass
import concourse.tile as tile
from concourse import mybir
from concourse.bass_utils import run_bass_kernel_spmd
from concourse.vector_clock import ScopedClock

B, H, T, D = 4, 16, 4096, 128
BS_EXPECTED = 32
C = T // BS_EXPECTED  # 128
N_CORES = 8
HEADS_PER_CORE = B * H // N_CORES  # 8
A = 32
N_SUB = 8  # 8 subs x 512 rows
F32 = mybir.dt.float32
BF16 = mybir.dt.bfloat16

# ---------------------------------------------------------------------------
# walrus in this toolchain rejects instructions carrying more than one sync
# wait; hoist extras onto same-engine NOPs.
_MAX_WAITS = 1
_split_counter = [0]


def _split_multi_waits(ordered):
    for insts in ordered.values():
        expanded = []
        for inst in insts:
            si = inst.sync_info
            if si is not None and len(si.on_wait) > _MAX_WAITS:
                waits = list(si.on_wait)
                head, keep = waits[:-_MAX_WAITS], waits[-_MAX_WAITS:]
                for w in head:
                    _split_counter[0] += 1
                    expanded.append(mybir.InstNoOp(
                        name=f"waitsplit_{_split_counter[0]}",
                        ins=[], outs=[],
                        engine=inst.engine,
                        sync_info=mybir.SyncInfo(on_wait=[w], on_update=[]),
                        bass_nofuse=True,
                    ))
                inst.sync_info = mybir.SyncInfo(
                    on_wait=keep, on_update=list(si.on_update)
                )
            expanded.append(inst)
        insts[:] = expanded


_orig_lower_ordered = tile.TileContext._lower_ordered_insts


def _lower_ordered_split(self, ordered):
    _split_multi_waits(ordered)
    return _orig_lower_ordered(self, ordered)


tile.TileContext._lower_ordered_insts = _lower_ordered_split


def _drain_and_barrier_split(self, tick_clock, wait_clock):
    nc = self.nc
    drain_inst = nc.sync.drain()
    wait_clock.add_sem_waits(
        drain_inst.ins, ScopedClock({None: tick_clock.global_clock})
    )
    si = drain_inst.ins.sync_info
    waits = list(si.on_wait) if si is not None else []
    if len(waits) > _MAX_WAITS:
        drain_inst.ins.sync_info = mybir.SyncInfo(
            on_wait=waits[:_MAX_WAITS], on_update=list(si.on_update)
        )
        for i in range(_MAX_WAITS, len(waits), _MAX_WAITS):
            extra = nc.sync.drain()
            extra.ins.sync_info = mybir.SyncInfo(
                on_wait=waits[i : i + _MAX_WAITS], on_update=[]
            )
    nc.all_engine_barrier()
    assert self.sems is not None
    popped = nc._tile_sem_poison_stack.pop()
    assert popped is self._sem_poison
    nc.clear_and_free_semaphores(list(self.sems.allocated().values()))
    nc.all_engine_barrier()


tile.TileContext._drain_and_barrier = _drain_and_barrier_split
# ---------------------------------------------------------------------------


def build_program(reps: int = 1, norm_engine: str = "dve",
                  store_engine: str = "scalar", qt_mode: str = "dma",
                  evac_mode: str = "fixed", norm_bcast: str = "y",
                  sums_mode: str = "head", mode: str = "full") -> bass.Bass:
    nc = bass.Bass("TRN2", target_bir_lowering=False, debug=False,
                   num_devices=N_CORES)

    q_d = nc.dram_tensor("q", [HEADS_PER_CORE, T, D], BF16,
                         kind="ExternalInput").ap()
    k_d = nc.dram_tensor("k", [HEADS_PER_CORE, T, D], BF16,
                         kind="ExternalInput").ap()
    v_d = nc.dram_tensor("v", [HEADS_PER_CORE, T, D], BF16,
                         kind="ExternalInput").ap()
    ident_d = nc.dram_tensor("ident", [128, 128], BF16,
                             kind="ExternalInput").ap()
    # scrambled outputs; host fixes layout
    outT_d = nc.dram_tensor("outT", [HEADS_PER_CORE, D, T], BF16,
                            kind="ExternalOutput").ap()
    score_d = nc.dram_tensor("score_scr", [HEADS_PER_CORE, 128, A, C], BF16,
                             kind="ExternalOutput").ap()

    exp_scale = 1.0 / (math.sqrt(D) * BS_EXPECTED)
    inv_bs = 1.0 / BS_EXPECTED

    def store_eng():
        return {"gpsimd": nc.gpsimd, "scalar": nc.scalar,
                "sync": nc.sync}[store_engine]

    with tile.TileContext(nc) as tc:
        with (
            tc.tile_pool(name="singles", bufs=1) as singles,
            tc.tile_pool(name="kv", bufs=4) as kv_pool,
            tc.tile_pool(name="qT", bufs=3) as qT_pool,
            tc.tile_pool(name="heads", bufs=2) as heads,
            tc.tile_pool(name="obuf", bufs=3) as obuf_pool,
            tc.tile_pool(name="w", bufs=3) as w_pool,
            tc.tile_pool(name="small", bufs=4) as small_pool,
            tc.tile_pool(name="psS", bufs=2, space="PSUM") as psS,
            tc.tile_pool(name="psT", bufs=2, space="PSUM") as psT,
            tc.tile_pool(name="psO", bufs=2, space="PSUM") as psO,
            tc.tile_pool(name="psC", bufs=1, space="PSUM") as psC,
        ):
            ident_bf = singles.tile([128, 128], BF16)
            nc.sync.dma_start(out=ident_bf, in_=ident_d)

            for _rep in range(reps):
                for h in range(HEADS_PER_CORE):
                    # ---- loads ------------------------------------------
                    qT_sb = qT_pool.tile([128, T], BF16, tag="qT")
                    if qt_mode == "dma" and mode != "compute":
                        nc.sync.dma_start_transpose(out=qT_sb, in_=q_d[h])
                    k_sb = kv_pool.tile([128, A, D], BF16, tag="kv")
                    v_sb = kv_pool.tile([128, A, D], BF16, tag="kv")
                    if mode != "compute":
                        nc.sync.dma_start(
                            out=k_sb,
                            in_=k_d[h].rearrange("(p a) d -> p a d", p=128),
                        )
                        nc.sync.dma_start(
                            out=v_sb,
                            in_=v_d[h].rearrange("(p a) d -> p a d", p=128),
                        )
                    if mode == "dma":
                        score_buf = obuf_pool.tile([128, A, C], BF16, tag="sc")
                        outT_buf = obuf_pool.tile([128, T], BF16, tag="ot")
                        touch = small_pool.tile([128, 4], BF16, tag="tc")
                        nc.vector.tensor_copy(touch[:, 0:1], k_sb[:, 0, 0:1])
                        nc.vector.tensor_copy(touch[:, 1:2], v_sb[:, 0, 0:1])
                        nc.vector.tensor_copy(touch[:, 2:3], qT_sb[:, 0:1])
                        nc.vector.memset(score_buf[:, 0:1, 0:1], 0.5)
                        nc.vector.memset(outT_buf[:, 0:1], 0.25)
                        store_eng().dma_start(out=score_d[h], in_=score_buf)
                        store_eng().dma_start(out=outT_d[h], in_=outT_buf)
                        continue

                    # ---- compression: 8 wide accumulating matmuls into
                    # partials [c, aa, d], then a strided DVE reduce -------
                    k_tmp = heads.tile([128, D], BF16, tag="kc")  # [c,d]*32
                    v_cmp = heads.tile([128, D], BF16, tag="vc")  # [c,d]*32
                    for src, dst in ((k_sb, k_tmp), (v_sb, v_cmp)):
                        cmp_ps = psC.tile([128, 4, 128], F32, tag="c")
                        flat = cmp_ps.rearrange("c aa d -> c (aa d)")
                        for g in range(8):
                            nc.tensor.matmul(
                                flat, lhsT=ident_bf,
                                rhs=src[:, 4 * g : 4 * (g + 1), :].rearrange(
                                    "p a d -> p (a d)"),
                                start=(g == 0), stop=(g == 7),
                            )
                        with nc.allow_low_precision(
                                reason="4-way partial sum to bf16; "
                                "tolerance 2e-2"):
                            nc.vector.reduce_sum(
                                dst, cmp_ps.rearrange("c aa d -> c d aa"),
                                axis=mybir.AxisListType.X,
                            )
                    kT_ps = psC.tile([128, 128], BF16, tag="ct")
                    nc.tensor.transpose(kT_ps, k_tmp, ident_bf)
                    k_cmpT = heads.tile([128, C], BF16, tag="kt")  # [d,c]
                    nc.scalar.copy(k_cmpT, kT_ps)

                    score_buf = obuf_pool.tile([128, A, C], BF16, tag="sc")
                    outT_buf = obuf_pool.tile([128, T], BF16, tag="ot")

                    # ---- main loop --------------------------------------
                    def do_qk_exp(s):
                        s_ps = psS.tile([128, 512], F32, tag="s")
                        for j in range(4):
                            nc.tensor.matmul(
                                s_ps[:, 128 * j : 128 * (j + 1)],
                                lhsT=qT_sb[:, 512 * s + 128 * j :
                                           512 * s + 128 * (j + 1)],
                                rhs=k_cmpT,
                                start=True, stop=True,
                            )
                        score_slots = score_buf.rearrange(
                            "p a c -> p (a c)")[:, 512 * s : 512 * (s + 1)]
                        nc.scalar.activation(
                            score_slots, s_ps,
                            mybir.ActivationFunctionType.Exp,
                            scale=exp_scale,
                        )

                    def do_norm(s, recip):
                        neng = (nc.gpsimd if norm_engine == "pool"
                                else nc.vector)
                        if norm_bcast == "y":
                            slots4 = score_buf[:, 4 * s : 4 * (s + 1), :]
                            neng.tensor_tensor(
                                slots4, slots4,
                                recip[:, :, None].broadcast_to((128, 4, C)),
                                op=mybir.AluOpType.mult,
                            )
                        else:
                            for j in range(4):
                                neng.tensor_scalar_mul(
                                    score_buf[:, 4 * s + j, :],
                                    score_buf[:, 4 * s + j, :],
                                    recip[:, j : j + 1],
                                )

                    def do_pv(s):
                        cols = slice(512 * s, 512 * (s + 1))
                        w_ps = psT.tile([128, 512], BF16, tag="w")
                        for j in range(4):
                            nc.tensor.transpose(
                                w_ps[:, 128 * j : 128 * (j + 1)],
                                score_buf[:, 4 * s + j, :], ident_bf,
                            )
                        w_sb = w_pool.tile([128, 512], BF16, tag="w")
                        if evac_mode == "alt" and s % 2 == 0:
                            nc.scalar.copy(w_sb, w_ps)
                        else:
                            nc.vector.tensor_copy(w_sb, w_ps)
                        o_ps = psO.tile([128, 512], F32, tag="o")
                        nc.tensor.matmul(
                            o_ps, lhsT=v_cmp, rhs=w_sb,
                            start=True, stop=True,
                        )
                        if evac_mode == "alt" and s % 2 == 0:
                            nc.vector.tensor_scalar_mul(
                                outT_buf[:, cols], o_ps, inv_bs)
                        else:
                            nc.scalar.activation(
                                outT_buf[:, cols], o_ps,
                                mybir.ActivationFunctionType.Copy,
                                scale=inv_bs,
                            )

                    if sums_mode == "head":
                        for s in range(N_SUB):
                            do_qk_exp(s)
                        sums_h = small_pool.tile([128, A], F32, tag="smh")
                        nc.vector.reduce_sum(
                            sums_h, score_buf, axis=mybir.AxisListType.X)
                        recip_h = small_pool.tile([128, A], F32, tag="rch")
                        nc.vector.reciprocal(recip_h, sums_h)
                        for s in range(N_SUB):
                            do_norm(s, recip_h[:, 4 * s : 4 * (s + 1)])
                            do_pv(s)
                    else:
                        for s in range(N_SUB):
                            do_qk_exp(s)
                            sums = small_pool.tile([128, 4], F32, tag="sm")
                            nc.vector.reduce_sum(
                                sums,
                                score_buf[:, 4 * s : 4 * (s + 1), :],
                                axis=mybir.AxisListType.X,
                            )
                            recip = small_pool.tile([128, 4], F32, tag="rc")
                            nc.vector.reciprocal(recip, sums)
                            do_norm(s, recip)
                            do_pv(s)

                    # ---- stores -----------------------------------------
                    if mode != "compute":
                        store_eng().dma_start(out=score_d[h], in_=score_buf)
                        store_eng().dma_start(out=outT_d[h], in_=outT_buf)
    return nc


def _make_const_inputs():
    import ml_dtypes
    return np.eye(128, dtype=ml_dtypes.bfloat16)


def make_in_maps(q, k, v):
    """q,k,v: [B*H, T, D] float32 -> per-core input dicts (bf16)."""
    import ml_dtypes
    qb = q.astype(ml_dtypes.bfloat16)
    kb = k.astype(ml_dtypes.bfloat16)
    vb = v.astype(ml_dtypes.bfloat16)
    ident = _make_const_inputs()
    in_maps = []
    for i in range(N_CORES):
        sl = slice(i * HEADS_PER_CORE, (i + 1) * HEADS_PER_CORE)
        in_maps.append({
            "q": qb[sl], "k": kb[sl], "v": vb[sl], "ident": ident,
        })
    return in_maps


def unscramble(res_list):
    """res_list: per-core dicts with outT [8,D,T] bf16, score_scr
    [8,128,A,C] bf16 -> (out [B,H,T,D] f32, score [B,H,T,C] f32)."""
    out = np.empty((B * H, T, D), dtype=np.float32)
    score = np.empty((B * H, T, C), dtype=np.float32)
    for i, res in enumerate(res_list):
        sl = slice(i * HEADS_PER_CORE, (i + 1) * HEADS_PER_CORE)
        ot = np.asarray(res["outT"], dtype=np.float32)  # [8, D, T]
        out[sl] = np.swapaxes(ot, 1, 2)
        sc = np.asarray(res["score_scr"], dtype=np.float32)  # [8,128,A,C]
        # score row t = 128*slot + p lives at [p, slot]
        score[sl] = np.swapaxes(sc, 1, 2).reshape(HEADS_PER_CORE, T, C)
    return out.reshape(B, H, T, D), score.reshape(B, H, T, C)


_PROGRAM_CACHE: dict[int, bass.Bass] = {}


def kernel(q: np.ndarray, k: np.ndarray, v: np.ndarray, BS) -> tuple:
    assert int(BS) == BS_EXPECTED, f"kernel hardcodes BS=32, got {BS}"
    q = np.ascontiguousarray(np.asarray(q, dtype=np.float32)).reshape(B * H, T, D)
    k = np.ascontiguousarray(np.asarray(k, dtype=np.float32)).reshape(B * H, T, D)
    v = np.ascontiguousarray(np.asarray(v, dtype=np.float32)).reshape(B * H, T, D)

    if 1 not in _PROGRAM_CACHE:
        _PROGRAM_CACHE[1] = build_program(reps=1)
    nc = _PROGRAM_CACHE[1]
    in_maps = make_in_maps(q, k, v)
    res = run_bass_kernel_spmd(nc, in_maps, core_ids=list(range(N_CORES)))
    return unscramble(res.results)
